# revision 6
# baseline (speedup 1.0000x reference)
"""Multi-head self-attention with RoPE on 8 Trainium2 NeuronCores.

Problem: B=2, S=2048, D=1024, H=16 heads, HD=64, causal, fp32.

Sharding: batch x head-group tensor parallel — core c owns batch c//4 and
heads 4*(c%4) .. 4*(c%4)+3 (two head-pairs). Each core computes its heads'
Q/K/V projections, RoPE, causal attention over its batch's 2048 tokens,
and a partial out-projection (W_out rows for its head features); the host
sums 4 partials per batch (bf16 on the wire) and adds b_out.

Per-core layout (feature-major = head-dim on partitions, tokens on free):
- q'/k' per head-pair: (128, 2048) bf16, rows = [hA d0..63 | hB d0..63]
- V: computed feature-major then PE-transposed into token-major blocks
  (128 tok, 256): [ones(64) | V_hA(64) | V_hB(64) | ones(64)]. PV matmul
  lhsT [ones|V_A] / [V_B|ones] makes PSUM carry both the attention
  numerator and the softmax denominator in one matmul; the middle 128
  columns are written by one contiguous ACT copy from a single PSUM bank
  holding all four 128x128 transposes of a 512-token chunk.
- scores computed transposed (kT on partitions, q on free); both heads'
  score matmuls row-pack the PE array (K=64 each). Diagonal blocks
  compute only the causally-live query columns (width 512-128m).
- phase-2 r-loop is software-pipelined: score(r+1) issues before PV(r)
  so the PE streams while ACT runs exp(r) (ACT is the phase-2 rate
  limiter at ~1.04us per 128-key block vs PE ~0.85us).
- RoPE: rotate-half via permutation matmul with the sin table
  sign-folded AND pre-permuted (spre), so the rotate term needs only a
  cheap bf16 SBUF multiply before the matmul instead of a second
  PSUM-input scalar_tensor_tensor afterwards.
- all matmuls bf16 (1 cycle/row), fp32 PSUM accumulate.
"""

import sys

if "/opt/trn_rl_repo" not in sys.path:
    sys.path.insert(0, "/opt/trn_rl_repo")

import numpy as np
import ml_dtypes

import concourse.bass as bass
import concourse.mybir as mybir
import concourse.tile as tile
from concourse import bacc
from concourse.bass_utils import run_bass_kernel_spmd

F32 = mybir.dt.float32
BF16 = mybir.dt.bfloat16
AF = mybir.ActivationFunctionType
ALU = mybir.AluOpType

B, S, D, H, HD = 2, 2048, 1024, 16, 64
T = B * S
NCORES = 8
GPB = NCORES // B              # head-groups per batch = 4
HPC = H // GPB                 # heads per core = 4 (2 pairs)
NP = HPC // 2                  # head pairs per core = 2
CW = HPC * HD                  # feature width per core = 256
ROPE_BASE = 10000.0
SCALE = 1.0 / np.sqrt(HD)

_CACHED = {}


def _mm(nc, out, lhsT, rhs, **kw):
    nc.tensor.matmul(out, lhsT, rhs, **kw)


def build_nc(reps=1, phases=(1, 2, 3)):
    nc = bacc.Bacc("TRN2", target_bir_lowering=False, debug=False,
                   num_devices=NCORES)

    qT = nc.dram_tensor("qT", [D, S], BF16, kind="ExternalInput")
    wq = nc.dram_tensor("wq", [D, CW], BF16, kind="ExternalInput")
    wk = nc.dram_tensor("wk", [D, CW], BF16, kind="ExternalInput")
    wv = nc.dram_tensor("wv", [D, CW], BF16, kind="ExternalInput")
    bq = nc.dram_tensor("bq", [128, NP], F32, kind="ExternalInput")
    bk = nc.dram_tensor("bk", [128, NP], F32, kind="ExternalInput")
    bv = nc.dram_tensor("bv", [128, NP], F32, kind="ExternalInput")
    cosT = nc.dram_tensor("cosT", [128, S], BF16, kind="ExternalInput")
    spreT = nc.dram_tensor("spreT", [128, S], BF16, kind="ExternalInput")
    tri = nc.dram_tensor("tri", [128, 128], BF16, kind="ExternalInput")
    rp = nc.dram_tensor("rp", [128, 128], BF16, kind="ExternalInput")
    eye = nc.dram_tensor("eye", [128, 128], BF16, kind="ExternalInput")
    wout = nc.dram_tensor("wout", [CW, D], BF16, kind="ExternalInput")
    outp = nc.dram_tensor("outp", [S, D], BF16, kind="ExternalOutput")

    KT = D // 128               # 8 contraction tiles

    with tile.TileContext(nc) as tc:
        with (
            tc.tile_pool(name="const", bufs=1) as cpool,
            tc.tile_pool(name="persist", bufs=1) as ppool,
        ):
            # ---- constants resident in SBUF ----
            wq_sb = cpool.tile([128, KT, CW], BF16)
            wk_sb = cpool.tile([128, KT, CW], BF16)
            wv_sb = cpool.tile([128, KT, CW], BF16)
            nc.sync.dma_start(wq_sb[:], wq[:].rearrange("(a p) f -> p a f", p=128))
            nc.sync.dma_start(wk_sb[:], wk[:].rearrange("(a p) f -> p a f", p=128))
            nc.sync.dma_start(wv_sb[:], wv[:].rearrange("(a p) f -> p a f", p=128))
            wout_sb = cpool.tile([128, CW // 128, D], BF16)
            nc.sync.dma_start(wout_sb[:],
                              wout[:].rearrange("(g p) f -> p g f", p=128))
            tri_sb = cpool.tile([128, 128], BF16)
            nc.sync.dma_start(tri_sb[:], tri[:])
            rp_sb = cpool.tile([128, 128], BF16)
            nc.sync.dma_start(rp_sb[:], rp[:])
            eye_sb = cpool.tile([128, 128], BF16)
            nc.sync.dma_start(eye_sb[:], eye[:])
            bq_sb = cpool.tile([128, NP], F32)
            bk_sb = cpool.tile([128, NP], F32)
            bv_sb = cpool.tile([128, NP], F32)
            nc.sync.dma_start(bq_sb[:], bq[:])
            nc.sync.dma_start(bk_sb[:], bk[:])
            nc.sync.dma_start(bv_sb[:], bv[:])
            cos_sb = cpool.tile([128, S], BF16)
            spre_sb = cpool.tile([128, S], BF16)
            nc.sync.dma_start(cos_sb[:], cosT[:])
            nc.sync.dma_start(spre_sb[:], spreT[:])

            # ---- persistent activations ----
            qf_t = [ppool.tile([128, NP, S // 2], BF16, name=f"qf{t}")
                    for t in range(2)]
            kf_t = [ppool.tile([128, NP, S // 2], BF16, name=f"kf{t}")
                    for t in range(2)]
            vt_t = [ppool.tile([128, NP, S // 256, 256], BF16, name=f"vt{t}")
                    for t in range(2)]
            at_sb = ppool.tile([128, NP, S], BF16)   # attn^T, stacked heads

            for t in range(2):
                nc.gpsimd.memset(vt_t[t][:, :, :, 0:64], 1.0)
                nc.gpsimd.memset(vt_t[t][:, :, :, 192:256], 1.0)

            for _rep in range(reps):
                _build_body(nc, tc, locals(), phases)

    nc.compile()
    return nc


def _build_body(nc, tc, env, phases=(1, 2, 3)):
    qT, outp = env["qT"], env["outp"]
    wq_sb, wk_sb, wv_sb = env["wq_sb"], env["wk_sb"], env["wv_sb"]
    wout_sb = env["wout_sb"]
    cos_sb, spre_sb = env["cos_sb"], env["spre_sb"]
    tri_sb, rp_sb, eye_sb = env["tri_sb"], env["rp_sb"], env["eye_sb"]
    bq_sb, bk_sb, bv_sb = env["bq_sb"], env["bk_sb"], env["bv_sb"]
    qf_t, kf_t = env["qf_t"], env["kf_t"]
    vt_t, at_sb = env["vt_t"], env["at_sb"]
    KT = env["KT"]

    # =========== phase 1: QKV projection + RoPE ===========
    if 1 in phases:
      with (
        tc.tile_pool(name="qt", bufs=17) as qtp,
        tc.tile_pool(name="raw", bufs=4) as rawp,
        tc.tile_pool(name="rs", bufs=4) as rsp,
        tc.tile_pool(name="vf", bufs=3) as vfp,
        tc.tile_pool(name="pmain", bufs=1, space="PSUM") as pmain,
        tc.tile_pool(name="prot", bufs=1, space="PSUM") as prot,
        tc.tile_pool(name="ptr", bufs=1, space="PSUM") as ptr,
      ):
        for tp in range(2):                      # 1024-token chunks
            tps = slice(1024 * tp, 1024 * (tp + 1))
            qts = []
            for kt in range(KT):
                qt_sb = qtp.tile([128, 1024], BF16, tag="qt", name=f"qt{kt}")
                qts.append(qt_sb)
                nc.sync.dma_start(qt_sb[:], qT[128 * kt:128 * (kt + 1), tps])
            for p in range(NP):                  # head pairs
                pf = slice(128 * p, 128 * (p + 1))
                ps_q = [pmain.tile([128, 512], F32, tag=f"psq{i}",
                                   name=f"psq{i}") for i in range(2)]
                ps_k = [pmain.tile([128, 512], F32, tag=f"psk{i}",
                                   name=f"psk{i}") for i in range(2)]
                ps_v = [pmain.tile([128, 512], F32, tag=f"psv{i}",
                                   name=f"psv{i}") for i in range(2)]
                for kt in range(KT):
                    for i in range(2):
                        hs = slice(512 * i, 512 * (i + 1))
                        _mm(nc, ps_q[i][:], wq_sb[:, kt, pf], qts[kt][:, hs],
                            start=(kt == 0), stop=(kt == KT - 1))
                        _mm(nc, ps_k[i][:], wk_sb[:, kt, pf], qts[kt][:, hs],
                            start=(kt == 0), stop=(kt == KT - 1))
                        _mm(nc, ps_v[i][:], wv_sb[:, kt, pf], qts[kt][:, hs],
                            start=(kt == 0), stop=(kt == KT - 1))

                for i in range(2):
                    ts = slice(512 * i, 512 * (i + 1))
                    gts = slice(1024 * tp + 512 * i, 1024 * tp + 512 * (i + 1))
                    for psx, fx, bx, rtag in (
                        (ps_q[i], qf_t[tp], bq_sb, "rq"),
                        (ps_k[i], kf_t[tp], bk_sb, "rk"),
                    ):
                        # raw = X + b (ACT); rs = raw * spre (DVE bf16 2x);
                        # rot term lands in PSUM via permutation matmul
                        raw = rawp.tile([128, 512], BF16, tag=rtag, name=rtag)
                        nc.scalar.activation(raw[:], psx[:], AF.Identity,
                                             bias=bx[:, p:p + 1])
                        rs = rsp.tile([128, 512], BF16, tag=rtag + "s",
                                      name=rtag + "s")
                        nc.vector.tensor_mul(rs[:], raw[:], spre_sb[:, gts])
                        ps_r = prot.tile([128, 512], F32, tag="rot",
                                         name="rot")
                        _mm(nc, ps_r[:], rp_sb[:], rs[:],
                            start=True, stop=True)
                        # fx = (X + b) * cos, then += rot-half term
                        nc.vector.scalar_tensor_tensor(
                            fx[:, p, ts], psx[:], bx[:, p:p + 1],
                            cos_sb[:, gts], ALU.add, ALU.mult)
                        nc.vector.tensor_add(fx[:, p, ts], fx[:, p, ts],
                                             ps_r[:])

                    # V: bias during ACT copy (feature-major), 4 PE
                    # transposes into ONE psum bank, one wide ACT drain
                    vf = vfp.tile([128, 512], BF16, tag="vf", name="vf")
                    nc.scalar.activation(vf[:], ps_v[i][:], AF.Identity,
                                         bias=bv_sb[:, p:p + 1])
                    ps_t4 = ptr.tile([128, 512], BF16, tag="pst", name="pst",
                                     padded_shape=[128, 1024])
                    for tt in range(4):
                        nc.tensor.matmul(
                            ps_t4[:, 128 * tt:128 * (tt + 1)],
                            vf[:, 128 * tt:128 * (tt + 1)], eye_sb[:],
                            is_transpose=True, start=(tt == 0),
                            stop=(tt == 3), skip_group_check=True)
                    nc.scalar.copy(vt_t[tp][:, p, 4 * i:4 * i + 4, 64:192],
                                   ps_t4[:])

    # =========== phase 2+3: attention + out-projection ===========
    with (
        tc.tile_pool(name="sps", bufs=3, space="PSUM") as sps,
        tc.tile_pool(name="aps", bufs=1, space="PSUM") as aps,
        tc.tile_pool(name="exppool", bufs=6) as expp,
        tc.tile_pool(name="recip", bufs=3) as rcpp,
        tc.tile_pool(name="ostage", bufs=6) as ostp,
    ):
        def make_oproj_emitters(c):
            """One closure per (token-tile, feature-half) chunk of the
            out-projection for chunk c's 512 tokens; injected one-per-r-step
            into the next c's r-loop so they fill PE slack under the
            ACT-bound exp stream."""
            emitters = []
            osbs = {}
            for tt in range(4 * c, 4 * c + 4):
                for nf in range(2):
                    def emit(tt=tt, nf=nf):
                        trows = slice(128 * tt, 128 * (tt + 1))
                        fs = slice(512 * nf, 512 * (nf + 1))
                        if nf == 0:
                            osbs[tt] = ostp.tile([128, 1024], BF16,
                                                 tag="ost", name="ost")
                        ps_o = pop.tile([128, 512], F32, tag="ps_o",
                                        name="ps_o")
                        for p in range(NP):
                            _mm(nc, ps_o[:], at_sb[:, p, trows],
                                wout_sb[:, p, fs],
                                start=(p == 0), stop=(p == NP - 1))
                        nc.vector.tensor_copy(osbs[tt][:, fs], ps_o[:])
                        if nf == 1:
                            nc.sync.dma_start(
                                outp[128 * tt:128 * (tt + 1), :],
                                osbs.pop(tt)[:])
                    emitters.append(emit)
            return emitters

        pending = []
        if 2 in phases:
          for c in range(4):
            for p in range(NP):
                cs = slice(512 * c, 512 * (c + 1))
                rmax = 4 * c + 3
                ph = [aps.tile([128, 512], F32, tag=f"pa{h}", name=f"pa{h}")
                      for h in range(2)]

                def emit_S(r, c=c, p=p):
                    ks_ = slice(128 * (r % 8), 128 * (r % 8) + 128)
                    w0 = 128 * max(r - 4 * c, 0)
                    ps_s = sps.tile([128, 1024], F32, tag="ps_s", name="ps_s")
                    for h in range(2):
                        p0 = 64 * h
                        _mm(nc, ps_s[:, 512 * h + w0:512 * (h + 1)],
                            kf_t[r // 8][p0:p0 + 64, p, ks_],
                            qf_t[c // 2][p0:p0 + 64, p,
                                         512 * (c % 2) + w0:512 * (c % 2) + 512],
                            start=True, stop=True)
                    return ps_s

                ps_cur = emit_S(0)
                for r in range(rmax + 1):
                    m = r - 4 * c
                    ps_nxt = emit_S(r + 1) if r < rmax else None
                    exp_sb = expp.tile([128, 1024], BF16, tag="exp",
                                       name="exp")
                    if m <= 0:
                        nc.scalar.activation(exp_sb[:], ps_cur[:], AF.Exp,
                                             scale=float(SCALE))
                    else:
                        # diagonal: only q-columns >= 128*m attend this
                        # block; one strided instr covers both heads
                        src3 = ps_cur[:].rearrange(
                            "p (a b) -> p a b", a=2)[:, :, 128 * m:512]
                        dst3 = exp_sb[:].rearrange(
                            "p (a b) -> p a b", a=2)[:, :, 128 * m:512]
                        nc.scalar.activation(dst3, src3, AF.Exp,
                                             scale=float(SCALE))
                    if m >= 0:  # triangle on the 128-col diagonal sub-block
                        for h in range(2):
                            so = 512 * h + 128 * m
                            nc.vector.tensor_mul(exp_sb[:, so:so + 128],
                                                 exp_sb[:, so:so + 128],
                                                 tri_sb[:])
                    mm_ = max(m, 0)
                    for h in range(2):
                        # hA: [ones|V_A] -> rows 0-63 sums, 64-127 attn
                        # hB: [V_B|ones] -> rows 0-63 attn, 64-127 sums
                        _mm(nc, ph[h][:, 128 * mm_:512],
                            vt_t[r // 8][:, p, r % 8, 128 * h:128 * (h + 1)],
                            exp_sb[:, 512 * h + 128 * mm_:512 * (h + 1)],
                            start=(r == 0), stop=(r == rmax))
                    if pending:
                        pending.pop(0)()
                    ps_cur = ps_nxt

                # normalize: at rows 0:63 = hB attn, 64:127 = hA attn
                # (wout rows are host-permuted [hB|hA] per pair to match)
                rc = rcpp.tile([128, 512], F32, tag="rc", name="rc")
                nc.vector.reciprocal(rc[0:64, :], ph[1][64:128, :])
                nc.vector.reciprocal(rc[64:128, :], ph[0][0:64, :])
                nc.vector.tensor_mul(at_sb[0:64, p, cs],
                                     ph[1][0:64, :], rc[0:64, :])
                nc.vector.tensor_mul(at_sb[64:128, p, cs],
                                     ph[0][64:128, :], rc[64:128, :])

            # queue this c's out-projection; flush any leftovers first
            if 3 not in phases:
                continue
            for f in pending:
                f()
            pending = make_oproj_emitters(c)
          for f in pending:
            f()


def _host_prep(query, W_qkv, b_qkv, W_out, b_out):
    """Build per-core input maps. Core c: batch c//GPB, head-group c%GPB."""
    query = np.asarray(query, dtype=np.float32)
    qTb = [np.ascontiguousarray(query[b].T) for b in range(B)]  # (D, S)

    inv_freq = 1.0 / (ROPE_BASE ** (np.arange(0, HD, 2, dtype=np.float32) / HD))
    freqs = np.arange(S, dtype=np.float32)[:, None] * inv_freq[None, :]
    emb = np.concatenate([freqs, freqs], axis=-1)          # (S, 64)
    cos = np.cos(emb).astype(np.float32).T                  # (64, S)
    sin = np.sin(emb).astype(np.float32).T
    sinp = sin.copy()
    sinp[0:32] = -sin[0:32]                                 # sign-folded
    # pre-permuted for the multiply-before-rotate order: spre[k] = sinp[swap(k)]
    spre = np.concatenate([sinp[32:64], sinp[0:32]], axis=0)
    cos128 = np.ascontiguousarray(np.tile(cos, (2, 1)))     # (128, S)
    spre128 = np.ascontiguousarray(np.tile(spre, (2, 1)))

    tri = np.ascontiguousarray(
        (np.arange(128)[None, :] >= np.arange(128)[:, None]).astype(np.float32))
    eye = np.eye(128, dtype=np.float32)
    # rotate-half permutation: rp[k, m] = 1 iff k == swap(m); swap exchanges
    # 32-halves within each 64-block
    rp = np.zeros((128, 128), dtype=np.float32)
    for h in range(2):
        for i in range(64):
            rp[64 * h + (i + 32) % 64, 64 * h + i] = 1.0

    W_qkv = np.asarray(W_qkv, dtype=np.float32)
    b_qkv = np.asarray(b_qkv, dtype=np.float32)
    W_out = np.asarray(W_out, dtype=np.float32)

    in_maps = []
    for c in range(NCORES):
        b = c // GPB
        g = c % GPB
        cols = slice(CW * g, CW * (g + 1))
        bqc = np.ascontiguousarray(b_qkv[0:D][cols].reshape(NP, 128).T)
        bkc = np.ascontiguousarray(b_qkv[D:2 * D][cols].reshape(NP, 128).T)
        bvc = np.ascontiguousarray(
            b_qkv[2 * D:3 * D][cols].reshape(NP, 128).T)
        # wout rows permuted [hB d0-63 | hA d0-63] per pair to match the
        # at_sb row order produced by the [ones|V_A]/[V_B|ones] PV layout
        wo = W_out[CW * g:CW * (g + 1), :].reshape(NP, 2, 64, D)
        wo = np.ascontiguousarray(wo[:, ::-1].reshape(CW, D))
        in_maps.append({
            "qT": qTb[b].astype(ml_dtypes.bfloat16),
            "wq": np.ascontiguousarray(W_qkv[:, 0:D][:, cols]).astype(ml_dtypes.bfloat16),
            "wk": np.ascontiguousarray(W_qkv[:, D:2 * D][:, cols]).astype(ml_dtypes.bfloat16),
            "wv": np.ascontiguousarray(W_qkv[:, 2 * D:3 * D][:, cols]).astype(ml_dtypes.bfloat16),
            "bq": bqc,
            "bk": bkc,
            "bv": bvc,
            "cosT": cos128.astype(ml_dtypes.bfloat16),
            "spreT": spre128.astype(ml_dtypes.bfloat16),
            "tri": tri.astype(ml_dtypes.bfloat16),
            "rp": rp.astype(ml_dtypes.bfloat16),
            "eye": eye.astype(ml_dtypes.bfloat16),
            "wout": wo.astype(ml_dtypes.bfloat16),
        })
    return in_maps


def kernel(query, W_qkv, b_qkv, W_out, b_out):
    if "nc" not in _CACHED:
        _CACHED["nc"] = build_nc()
    nc = _CACHED["nc"]
    in_maps = _host_prep(query, W_qkv, b_qkv, W_out, b_out)
    res = run_bass_kernel_spmd(nc, in_maps, core_ids=list(range(NCORES)))
    acc = np.zeros((B, S, D), dtype=np.float64)
    for c, r in enumerate(res.results):
        acc[c // GPB] += np.asarray(r["outp"], dtype=np.float64)
    acc += np.asarray(b_out, dtype=np.float64)[None, None, :]
    return acc.astype(np.float32)


# revision 13
# speedup vs baseline: 1.1392x; 1.1392x over previous
"""Multi-head self-attention with RoPE on 8 Trainium2 NeuronCores.

Problem: B=2, S=2048, D=1024, H=16 heads, HD=64, causal, fp32.

Sharding: batch x head-group tensor parallel — core c owns batch c//4 and
heads 4*(c%4) .. 4*(c%4)+3 (two head-pairs). Each core computes its heads'
Q/K/V projections, RoPE, causal attention over its batch's 2048 tokens,
and a partial out-projection (W_out rows for its head features); the host
sums 4 partials per batch (bf16 on the wire) and adds b_out.

Per-core layout (feature-major = head-dim on partitions, tokens on free):
- q'/k' per head-pair: (128, 2048) bf16, rows = [hA d0..63 | hB d0..63]
- V: computed feature-major then PE-transposed into token-major blocks
  (128 tok, 256): [ones(64) | V_hA(64) | V_hB(64) | ones(64)]. PV matmul
  lhsT [ones|V_A] / [V_B|ones] makes PSUM carry both the attention
  numerator and the softmax denominator in one matmul; the middle 128
  columns are written by one contiguous ACT copy from a single PSUM bank
  holding all four 128x128 transposes of a 512-token chunk.
- scores computed transposed (kT on partitions, q on free); both heads'
  score matmuls row-pack the PE array (K=64 each). Diagonal blocks
  compute only the causally-live query columns (width 512-128m).
- phase-2 r-loop is software-pipelined: score(r+1) issues before PV(r)
  so the PE streams while ACT runs exp(r) (ACT is the phase-2 rate
  limiter at ~1.04us per 128-key block vs PE ~0.85us).
- RoPE: rotate-half via permutation matmul with the sin table
  sign-folded AND pre-permuted (spre), so the rotate term needs only a
  cheap bf16 SBUF multiply before the matmul instead of a second
  PSUM-input scalar_tensor_tensor afterwards.
- all matmuls bf16 (1 cycle/row), fp32 PSUM accumulate.
"""

import sys

if "/opt/trn_rl_repo" not in sys.path:
    sys.path.insert(0, "/opt/trn_rl_repo")

import numpy as np
import ml_dtypes

import concourse.bass as bass
import concourse.mybir as mybir
import concourse.tile as tile
from concourse import bacc
from concourse.bass_utils import run_bass_kernel_spmd

F32 = mybir.dt.float32
BF16 = mybir.dt.bfloat16
AF = mybir.ActivationFunctionType
ALU = mybir.AluOpType

B, S, D, H, HD = 2, 2048, 1024, 16, 64
T = B * S
NCORES = 8
GPB = NCORES // B              # head-groups per batch = 4
HPC = H // GPB                 # heads per core = 4 (2 pairs)
NP = HPC // 2                  # head pairs per core = 2
CW = HPC * HD                  # feature width per core = 256
ROPE_BASE = 10000.0
SCALE = 1.0 / np.sqrt(HD)

_CACHED = {}


def _mm(nc, out, lhsT, rhs, **kw):
    nc.tensor.matmul(out, lhsT, rhs, **kw)


def build_nc(reps=1, phases=(1, 2, 3)):
    nc = bacc.Bacc("TRN2", target_bir_lowering=False, debug=False,
                   num_devices=NCORES)

    qT = nc.dram_tensor("qT", [D, S], BF16, kind="ExternalInput")
    wq = nc.dram_tensor("wq", [D, CW], BF16, kind="ExternalInput")
    wk = nc.dram_tensor("wk", [D, CW], BF16, kind="ExternalInput")
    wv = nc.dram_tensor("wv", [D, CW], BF16, kind="ExternalInput")
    bq = nc.dram_tensor("bq", [128, NP], F32, kind="ExternalInput")
    bk = nc.dram_tensor("bk", [128, NP], F32, kind="ExternalInput")
    bv = nc.dram_tensor("bv", [128, NP], F32, kind="ExternalInput")
    cosT = nc.dram_tensor("cosT", [128, S], BF16, kind="ExternalInput")
    spreT = nc.dram_tensor("spreT", [128, S], BF16, kind="ExternalInput")
    tri = nc.dram_tensor("tri", [128, 128], BF16, kind="ExternalInput")
    rp = nc.dram_tensor("rp", [128, 128], BF16, kind="ExternalInput")
    eye = nc.dram_tensor("eye", [128, 128], BF16, kind="ExternalInput")
    wout = nc.dram_tensor("wout", [CW, D], BF16, kind="ExternalInput")
    outp = nc.dram_tensor("outp", [S, D], BF16, kind="ExternalOutput")

    KT = D // 128               # 8 contraction tiles

    with tile.TileContext(nc) as tc:
        with (
            tc.tile_pool(name="const", bufs=1) as cpool,
            tc.tile_pool(name="persist", bufs=1) as ppool,
            tc.tile_pool(name="qt", bufs=17) as qtp,
        ):
            # ---- constants resident in SBUF ----
            wq_sb = cpool.tile([128, KT, CW], BF16)
            wk_sb = cpool.tile([128, KT, CW], BF16)
            wv_sb = cpool.tile([128, KT, CW], BF16)
            nc.sync.dma_start(wq_sb[:], wq[:].rearrange("(a p) f -> p a f", p=128))
            nc.sync.dma_start(wk_sb[:], wk[:].rearrange("(a p) f -> p a f", p=128))
            nc.sync.dma_start(wv_sb[:], wv[:].rearrange("(a p) f -> p a f", p=128))
            wout_sb = cpool.tile([128, CW // 128, D], BF16)
            nc.sync.dma_start(wout_sb[:],
                              wout[:].rearrange("(g p) f -> p g f", p=128))
            tri_sb = cpool.tile([128, 128], BF16)
            nc.sync.dma_start(tri_sb[:], tri[:])
            rp_sb = cpool.tile([128, 128], BF16)
            nc.sync.dma_start(rp_sb[:], rp[:])
            eye_sb = cpool.tile([128, 128], BF16)
            nc.sync.dma_start(eye_sb[:], eye[:])
            bq_sb = cpool.tile([128, NP], F32)
            bk_sb = cpool.tile([128, NP], F32)
            bv_sb = cpool.tile([128, NP], F32)
            nc.sync.dma_start(bq_sb[:], bq[:])
            nc.sync.dma_start(bk_sb[:], bk[:])
            nc.sync.dma_start(bv_sb[:], bv[:])
            cos_sb = cpool.tile([128, S], BF16)
            spre_sb = cpool.tile([128, S], BF16)
            nc.sync.dma_start(cos_sb[:], cosT[:])
            nc.sync.dma_start(spre_sb[:], spreT[:])

            # ---- persistent activations (double-buffered by rep parity
            # so rep N+1's projections never wait on rep N's attention) ----
            qf_b = [[ppool.tile([128, NP, S // 2], BF16, name=f"qf{u}{t}")
                     for t in range(2)] for u in range(2)]
            kf_b = [[ppool.tile([128, NP, S // 2], BF16, name=f"kf{u}{t}")
                     for t in range(2)] for u in range(2)]
            vt_b = [[ppool.tile([128, NP, S // 256, 256], BF16,
                                name=f"vt{u}{t}")
                     for t in range(2)] for u in range(2)]
            at_sb = ppool.tile([128, NP, S], BF16)   # attn^T, stacked heads

            for u in range(2):
                for t in range(2):
                    nc.gpsimd.memset(vt_b[u][t][:, :, :, 0:64], 1.0)
                    nc.gpsimd.memset(vt_b[u][t][:, :, :, 192:256], 1.0)

            qts_cur = _emit_qt_loads(nc, qtp, qT, KT)
            op_pending = []
            for _rep in range(reps):
                qf_t, kf_t, vt_t = (qf_b[_rep % 2], kf_b[_rep % 2],
                                    vt_b[_rep % 2])
                last_rep = _rep == reps - 1
                qts_cur = _build_body(nc, tc, locals(), phases)

    nc.compile()
    return nc


def _emit_qt_loads(nc, qtp, qT, KT):
    """DMA the full qT for one rep into SBUF tiles. Called one rep ahead
    (at the previous rep's phase-2 emission point) so the loads run on the
    SP queue during attention, before that rep's outp stores."""
    import concourse.tile as _t  # noqa: F401  (kept for parity)
    qts = []
    for tp in range(2):
        tps = slice(1024 * tp, 1024 * (tp + 1))
        row = []
        for kt in range(KT):
            qt_sb = qtp.tile([128, 1024], BF16, tag="qt",
                             name=f"qt{tp}{kt}")
            row.append(qt_sb)
            nc.sync.dma_start(qt_sb[:], qT[128 * kt:128 * (kt + 1), tps])
        qts.append(row)
    return qts


def _build_body(nc, tc, env, phases=(1, 2, 3)):
    qT, outp = env["qT"], env["outp"]
    wq_sb, wk_sb, wv_sb = env["wq_sb"], env["wk_sb"], env["wv_sb"]
    wout_sb = env["wout_sb"]
    cos_sb, spre_sb = env["cos_sb"], env["spre_sb"]
    tri_sb, rp_sb, eye_sb = env["tri_sb"], env["rp_sb"], env["eye_sb"]
    bq_sb, bk_sb, bv_sb = env["bq_sb"], env["bk_sb"], env["bv_sb"]
    qf_t, kf_t = env["qf_t"], env["kf_t"]
    vt_t, at_sb = env["vt_t"], env["at_sb"]
    KT = env["KT"]
    qtp, qts_cur, last_rep = env["qtp"], env["qts_cur"], env["last_rep"]

    # =========== phase 1: QKV projection + RoPE ===========
    if 1 in phases:
      with (
        tc.tile_pool(name="raw", bufs=4) as rawp,
        tc.tile_pool(name="rs", bufs=4) as rsp,
        tc.tile_pool(name="vf", bufs=3) as vfp,
        tc.tile_pool(name="pmain", bufs=1, space="PSUM") as pmain,
        tc.tile_pool(name="prot", bufs=1, space="PSUM") as prot,
        tc.tile_pool(name="ptr", bufs=1, space="PSUM") as ptr,
      ):
        for tp in range(2):                      # 1024-token chunks
            qts = qts_cur[tp]
            for p in range(NP):                  # head pairs
                pf = slice(128 * p, 128 * (p + 1))
                ps_q = [pmain.tile([128, 512], F32, tag=f"psq{i}",
                                   name=f"psq{i}") for i in range(2)]
                ps_k = [pmain.tile([128, 512], F32, tag=f"psk{i}",
                                   name=f"psk{i}") for i in range(2)]
                ps_v = [pmain.tile([128, 512], F32, tag=f"psv{i}",
                                   name=f"psv{i}") for i in range(2)]
                for kt in range(KT):
                    for i in range(2):
                        hs = slice(512 * i, 512 * (i + 1))
                        _mm(nc, ps_q[i][:], wq_sb[:, kt, pf], qts[kt][:, hs],
                            start=(kt == 0), stop=(kt == KT - 1))
                        _mm(nc, ps_k[i][:], wk_sb[:, kt, pf], qts[kt][:, hs],
                            start=(kt == 0), stop=(kt == KT - 1))
                        _mm(nc, ps_v[i][:], wv_sb[:, kt, pf], qts[kt][:, hs],
                            start=(kt == 0), stop=(kt == KT - 1))

                for i in range(2):
                    ts = slice(512 * i, 512 * (i + 1))
                    gts = slice(1024 * tp + 512 * i, 1024 * tp + 512 * (i + 1))
                    for psx, fx, bx, rtag in (
                        (ps_q[i], qf_t[tp], bq_sb, "rq"),
                        (ps_k[i], kf_t[tp], bk_sb, "rk"),
                    ):
                        # raw = X + b (ACT); rs = raw * spre (DVE bf16 2x);
                        # rot term lands in PSUM via permutation matmul
                        raw = rawp.tile([128, 512], BF16, tag=rtag, name=rtag)
                        nc.scalar.activation(raw[:], psx[:], AF.Identity,
                                             bias=bx[:, p:p + 1])
                        rs = rsp.tile([128, 512], BF16, tag=rtag + "s",
                                      name=rtag + "s")
                        nc.vector.tensor_mul(rs[:], raw[:], spre_sb[:, gts])
                        ps_r = prot.tile([128, 512], F32, tag="rot",
                                         name="rot")
                        _mm(nc, ps_r[:], rp_sb[:], rs[:],
                            start=True, stop=True)
                        # fx = (X + b) * cos, then += rot-half term
                        nc.vector.scalar_tensor_tensor(
                            fx[:, p, ts], psx[:], bx[:, p:p + 1],
                            cos_sb[:, gts], ALU.add, ALU.mult)
                        nc.vector.tensor_add(fx[:, p, ts], fx[:, p, ts],
                                             ps_r[:])

                    # V: bias during ACT copy (feature-major), 4 PE
                    # transposes into ONE psum bank, one wide ACT drain
                    vf = vfp.tile([128, 512], BF16, tag="vf", name="vf")
                    nc.scalar.activation(vf[:], ps_v[i][:], AF.Identity,
                                         bias=bv_sb[:, p:p + 1])
                    ps_t4 = ptr.tile([128, 512], BF16, tag="pst", name="pst",
                                     padded_shape=[128, 1024])
                    for tt in range(4):
                        nc.tensor.matmul(
                            ps_t4[:, 128 * tt:128 * (tt + 1)],
                            vf[:, 128 * tt:128 * (tt + 1)], eye_sb[:],
                            is_transpose=True, start=(tt == 0),
                            stop=(tt == 3), skip_group_check=True)
                    nc.scalar.copy(vt_t[tp][:, p, 4 * i:4 * i + 4, 64:192],
                                   ps_t4[:])

    # =========== phase 2+3: attention + out-projection ===========
    qts_next = None if last_rep else _emit_qt_loads(nc, qtp, qT, KT)

    with (
        tc.tile_pool(name="sps", bufs=3, space="PSUM") as sps,
        tc.tile_pool(name="aps", bufs=1, space="PSUM") as aps,
        tc.tile_pool(name="exppool", bufs=6) as expp,
        tc.tile_pool(name="recip", bufs=3) as rcpp,
        tc.tile_pool(name="ostage", bufs=6) as ostp,
    ):
        def make_oproj_emitters(c):
            """One closure per (token-tile, feature-half) chunk of the
            out-projection for chunk c's 512 tokens; injected one-per-r-step
            into the next c's r-loop (crossing the rep boundary for c=3) so
            they fill PE slack under the ACT-bound exp stream. Pools are
            passed at call time since the emitting body's pools may have
            been released."""
            emitters = []
            osbs = {}
            for tt in range(4 * c, 4 * c + 4):
                for nf in range(2):
                    def emit(sps, ostp, tt=tt, nf=nf):
                        trows = slice(128 * tt, 128 * (tt + 1))
                        fs = slice(512 * nf, 512 * (nf + 1))
                        if nf == 0:
                            osbs[tt] = ostp.tile([128, 1024], BF16,
                                                 tag="ost", name="ost")
                        pso = sps.tile([128, 1024], F32, tag="ps_s",
                                       name="ps_o")
                        ps_o = pso[:, 0:512]
                        for p in range(NP):
                            _mm(nc, ps_o, at_sb[:, p, trows],
                                wout_sb[:, p, fs],
                                start=(p == 0), stop=(p == NP - 1))
                        nc.vector.tensor_copy(osbs[tt][:, fs], ps_o)
                        if nf == 1:
                            nc.sync.dma_start(
                                outp[128 * tt:128 * (tt + 1), :],
                                osbs.pop(tt)[:])
                    emitters.append(emit)
            return emitters

        pending = env["op_pending"]
        if 2 in phases:
          for c in range(4):
            for p in range(NP):
                cs = slice(512 * c, 512 * (c + 1))
                rmax = 4 * c + 3
                ph = [aps.tile([128, 512], F32, tag=f"pa{h}", name=f"pa{h}")
                      for h in range(2)]

                def emit_S(r, c=c, p=p):
                    ks_ = slice(128 * (r % 8), 128 * (r % 8) + 128)
                    w0 = 128 * max(r - 4 * c, 0)
                    ps_s = sps.tile([128, 1024], F32, tag="ps_s", name="ps_s")
                    for h in range(2):
                        p0 = 64 * h
                        _mm(nc, ps_s[:, 512 * h + w0:512 * (h + 1)],
                            kf_t[r // 8][p0:p0 + 64, p, ks_],
                            qf_t[c // 2][p0:p0 + 64, p,
                                         512 * (c % 2) + w0:512 * (c % 2) + 512],
                            start=True, stop=True)
                    return ps_s

                ps_cur = emit_S(0)
                for r in range(rmax + 1):
                    m = r - 4 * c
                    ps_nxt = emit_S(r + 1) if r < rmax else None
                    exp_sb = expp.tile([128, 1024], BF16, tag="exp",
                                       name="exp")
                    if m <= 0:
                        nc.scalar.activation(exp_sb[:], ps_cur[:], AF.Exp,
                                             scale=float(SCALE))
                    else:
                        # diagonal: only q-columns >= 128*m attend this
                        # block; one strided instr covers both heads
                        src3 = ps_cur[:].rearrange(
                            "p (a b) -> p a b", a=2)[:, :, 128 * m:512]
                        dst3 = exp_sb[:].rearrange(
                            "p (a b) -> p a b", a=2)[:, :, 128 * m:512]
                        nc.scalar.activation(dst3, src3, AF.Exp,
                                             scale=float(SCALE))
                    if m >= 0:  # triangle on the 128-col diagonal sub-block
                        for h in range(2):
                            so = 512 * h + 128 * m
                            nc.vector.tensor_mul(exp_sb[:, so:so + 128],
                                                 exp_sb[:, so:so + 128],
                                                 tri_sb[:])
                    mm_ = max(m, 0)
                    for h in range(2):
                        # hA: [ones|V_A] -> rows 0-63 sums, 64-127 attn
                        # hB: [V_B|ones] -> rows 0-63 attn, 64-127 sums
                        _mm(nc, ph[h][:, 128 * mm_:512],
                            vt_t[r // 8][:, p, r % 8, 128 * h:128 * (h + 1)],
                            exp_sb[:, 512 * h + 128 * mm_:512 * (h + 1)],
                            start=(r == 0), stop=(r == rmax))
                    if pending:
                        pending.pop(0)(sps, ostp)
                    ps_cur = ps_nxt

                # normalize: at rows 0:63 = hB attn, 64:127 = hA attn
                # (wout rows are host-permuted [hB|hA] per pair to match)
                rc = rcpp.tile([128, 512], F32, tag="rc", name="rc")
                nc.vector.reciprocal(rc[0:64, :], ph[1][64:128, :])
                nc.vector.reciprocal(rc[64:128, :], ph[0][0:64, :])
                nc.vector.tensor_mul(at_sb[0:64, p, cs],
                                     ph[1][0:64, :], rc[0:64, :])
                nc.vector.tensor_mul(at_sb[64:128, p, cs],
                                     ph[0][64:128, :], rc[64:128, :])

            # queue this c's out-projection; flush any leftovers first
            if 3 not in phases:
                continue
            for f in pending:
                f(sps, ostp)
            pending.clear()
            pending.extend(make_oproj_emitters(c))
          if last_rep:
            for f in pending:
                f(sps, ostp)
            pending.clear()
    return qts_next


def _host_prep(query, W_qkv, b_qkv, W_out, b_out):
    """Build per-core input maps. Core c: batch c//GPB, head-group c%GPB."""
    query = np.asarray(query, dtype=np.float32)
    qTb = [np.ascontiguousarray(query[b].T) for b in range(B)]  # (D, S)

    inv_freq = 1.0 / (ROPE_BASE ** (np.arange(0, HD, 2, dtype=np.float32) / HD))
    freqs = np.arange(S, dtype=np.float32)[:, None] * inv_freq[None, :]
    emb = np.concatenate([freqs, freqs], axis=-1)          # (S, 64)
    cos = np.cos(emb).astype(np.float32).T                  # (64, S)
    sin = np.sin(emb).astype(np.float32).T
    sinp = sin.copy()
    sinp[0:32] = -sin[0:32]                                 # sign-folded
    # pre-permuted for the multiply-before-rotate order: spre[k] = sinp[swap(k)]
    spre = np.concatenate([sinp[32:64], sinp[0:32]], axis=0)
    cos128 = np.ascontiguousarray(np.tile(cos, (2, 1)))     # (128, S)
    spre128 = np.ascontiguousarray(np.tile(spre, (2, 1)))

    tri = np.ascontiguousarray(
        (np.arange(128)[None, :] >= np.arange(128)[:, None]).astype(np.float32))
    eye = np.eye(128, dtype=np.float32)
    # rotate-half permutation: rp[k, m] = 1 iff k == swap(m); swap exchanges
    # 32-halves within each 64-block
    rp = np.zeros((128, 128), dtype=np.float32)
    for h in range(2):
        for i in range(64):
            rp[64 * h + (i + 32) % 64, 64 * h + i] = 1.0

    W_qkv = np.asarray(W_qkv, dtype=np.float32)
    b_qkv = np.asarray(b_qkv, dtype=np.float32)
    W_out = np.asarray(W_out, dtype=np.float32)

    in_maps = []
    for c in range(NCORES):
        b = c // GPB
        g = c % GPB
        cols = slice(CW * g, CW * (g + 1))
        bqc = np.ascontiguousarray(b_qkv[0:D][cols].reshape(NP, 128).T)
        bkc = np.ascontiguousarray(b_qkv[D:2 * D][cols].reshape(NP, 128).T)
        bvc = np.ascontiguousarray(
            b_qkv[2 * D:3 * D][cols].reshape(NP, 128).T)
        # wout rows permuted [hB d0-63 | hA d0-63] per pair to match the
        # at_sb row order produced by the [ones|V_A]/[V_B|ones] PV layout
        wo = W_out[CW * g:CW * (g + 1), :].reshape(NP, 2, 64, D)
        wo = np.ascontiguousarray(wo[:, ::-1].reshape(CW, D))
        in_maps.append({
            "qT": qTb[b].astype(ml_dtypes.bfloat16),
            "wq": np.ascontiguousarray(W_qkv[:, 0:D][:, cols]).astype(ml_dtypes.bfloat16),
            "wk": np.ascontiguousarray(W_qkv[:, D:2 * D][:, cols]).astype(ml_dtypes.bfloat16),
            "wv": np.ascontiguousarray(W_qkv[:, 2 * D:3 * D][:, cols]).astype(ml_dtypes.bfloat16),
            "bq": bqc,
            "bk": bkc,
            "bv": bvc,
            "cosT": cos128.astype(ml_dtypes.bfloat16),
            "spreT": spre128.astype(ml_dtypes.bfloat16),
            "tri": tri.astype(ml_dtypes.bfloat16),
            "rp": rp.astype(ml_dtypes.bfloat16),
            "eye": eye.astype(ml_dtypes.bfloat16),
            "wout": wo.astype(ml_dtypes.bfloat16),
        })
    return in_maps


def kernel(query, W_qkv, b_qkv, W_out, b_out):
    if "nc" not in _CACHED:
        _CACHED["nc"] = build_nc()
    nc = _CACHED["nc"]
    in_maps = _host_prep(query, W_qkv, b_qkv, W_out, b_out)
    res = run_bass_kernel_spmd(nc, in_maps, core_ids=list(range(NCORES)))
    acc = np.zeros((B, S, D), dtype=np.float64)
    for c, r in enumerate(res.results):
        acc[c // GPB] += np.asarray(r["outp"], dtype=np.float64)
    acc += np.asarray(b_out, dtype=np.float64)[None, None, :]
    return acc.astype(np.float32)
